# revision 1
# baseline (speedup 1.0000x reference)
"""Chamfer distance (bidirectional NN min-squared-distance) on 8 Trainium2 cores.

Strategy
--------
reference computes, per batch b (4 batches):
    dist1[b, i] = min_j ||xyz1[b,i] - xyz2[b,j]||^2      (16384 queries vs 16384 refs)
    dist2[b, j] = min_i ||xyz2[b,j] - xyz1[b,i]||^2
That is 8 independent "NN-min" jobs (4 batches x 2 directions) -> one job per
NeuronCore (SPMD, identical program, different data).

Per job, both point sets are sorted by x on the host. The device computes, for
each 128-query block t, squared distances only against a contiguous rank band
of W refs centered on the block diagonal (C(t) = 128t + 64 - W/2, clamped),
then takes a free-dim min-reduce on the vector engine. The squared distance is
expressed as a K=30 bf16 matmul: each point is lifted to 5 features
  queries:  (x, y, z, |q|^2, 1)        refs: (-2x, -2y, -2z, 1, |r|^2)
so d(i,j) = fa_i . gb_j, and every feature is split into three bf16 limbs
(hi/mid/lo) with the 6 significant cross-products kept, giving ~1e-5 absolute
accuracy (fp32-class) at full bf16 PE rate.

The band is a heuristic; exactness is restored on the host: a query's band-min
is provably the global min when it is below the squared x-gap to both band
edges (all excluded refs differ in x by at least that gap). The few queries
that fail this certificate (typically ~1-13%) are recomputed exactly with
numpy against all refs.
"""

import numpy as np
import ml_dtypes

import concourse.mybir as mybir
from concourse import bacc
from concourse.bass import ds
from concourse.bass_utils import run_bass_kernel_spmd
from concourse.expressions_rust import smax, smin
from concourse.ordered_set import OrderedSet

N = 16384
P = 128                  # partition block of queries
NBLK = N // P            # 128 query blocks
W = 1536                 # ref band width per block (multiple of 512)
KF = 5                   # base features
K = 6 * KF               # bf16 triple-split term groups
E_DEV = 3.0e-5           # conservative device abs-error bound used by the certificate

_CACHE = {}

# ----------------------------------------------------------------- device ---

def _build_nc():
    """Raw-bass program (no TileContext): one hardware Fori loop over the 128
    query blocks with hand-rolled counting semaphores, ~51 BIR instructions
    total.  The measured dispatch cost of this problem is dominated by
    per-call BIR->NEFF compile/serialize time, which scales with program
    instruction count (~50-90us/instruction), so a small dynamic loop beats
    the 450-instruction unrolled program by ~10x wall time even though the
    device executes the same work.

    Per iteration t: the Pool queue stages query block t into a fixed SBUF
    tile (ldweights needs a static offset), the PE runs 3 bf16 matmuls of the
    block against the register-clamped rank band c0(t), and the DVE
    min-reduces the PSUM band into strip[:, t].  Monotonic counting
    semaphores (DMA completions count in units of 16) chain
    stage->matmul->reduce and the WAR hazards between iterations; no
    per-iteration barrier or semaphore reset is needed."""
    ENG = OrderedSet([mybir.EngineType.PE, mybir.EngineType.DVE,
                      mybir.EngineType.Pool])
    nc = bacc.Bacc("TRN2", target_bir_lowering=False, debug=False)
    aT = nc.dram_tensor("aT", [K, N], mybir.dt.bfloat16, kind="ExternalInput").ap()
    gT = nc.dram_tensor("gT", [K, N], mybir.dt.bfloat16, kind="ExternalInput").ap()
    md = nc.dram_tensor("md", [P, NBLK], mybir.dt.float32, kind="ExternalOutput").ap()

    a_sb = nc.alloc_sbuf_tensor("a_sb", [K, N], mybir.dt.bfloat16).ap()
    g_sb = nc.alloc_sbuf_tensor("g_sb", [K, N], mybir.dt.bfloat16).ap()
    stage = nc.alloc_sbuf_tensor("stage", [K, P], mybir.dt.bfloat16).ap()
    strip = nc.alloc_sbuf_tensor("strip", [P, NBLK], mybir.dt.float32).ap()
    ps = nc.alloc_psum_tensor("ps", [P, W], mybir.dt.float32).ap()

    s_in = nc.alloc_semaphore("s_in")
    s_stage = nc.alloc_semaphore("s_stage")
    s_mm = nc.alloc_semaphore("s_mm")
    s_red = nc.alloc_semaphore("s_red")
    s_out = nc.alloc_semaphore("s_out")

    nc.gpsimd.dma_start(a_sb[:], aT[:, :]).then_inc(s_in, 16)
    nc.gpsimd.dma_start(g_sb[:], gT[:, :]).then_inc(s_in, 16)
    nc.gpsimd.wait_ge(s_in, 32)

    with nc.Fori(0, NBLK, 1, engines=ENG) as t:
        q0 = t * P
        c0 = smin(smax(q0 + (P // 2 - W // 2), 0), N - W)
        # Pool: stage query block t (WAR: matmuls of t-1 must have consumed)
        nc.gpsimd.wait_ge(s_mm, t)
        nc.gpsimd.dma_start(stage[:], a_sb[:, ds(q0, P)]).then_inc(s_stage, 16)
        # PE
        nc.tensor.wait_ge(s_stage, 16 * t + 16)
        nc.tensor.wait_ge(s_red, t)          # ps WAR vs reduce of t-1
        for c in range(W // 512):
            mm = nc.tensor.matmul(
                ps[:, c * 512:(c + 1) * 512],
                lhsT=stage[:],
                rhs=g_sb[:, ds(c0 + c * 512, 512)],
                start=True, stop=True,
            )
        mm.then_inc(s_mm, 1)
        # DVE
        nc.vector.wait_ge(s_mm, t + 1)
        nc.vector.tensor_reduce(
            out=strip[:, ds(t, 1)], in_=ps[:],
            axis=mybir.AxisListType.X, op=mybir.AluOpType.min,
        ).then_inc(s_red, 1)

    nc.gpsimd.wait_ge(s_red, NBLK)
    nc.gpsimd.dma_start(md[:, :], strip[:]).then_inc(s_out, 16)
    nc.gpsimd.wait_ge(s_out, 16)
    nc.finalize()
    return nc


def _get_nc():
    if "nc" not in _CACHE:
        _CACHE["nc"] = _build_nc()
    return _CACHE["nc"]

# ------------------------------------------------------------------- host ---

def _split3(f32):
    """fp32 array -> 3 bf16 limbs (hi, mid, lo), f ~= h + m + l."""
    h = f32.astype(ml_dtypes.bfloat16)
    r = f32 - h.astype(np.float32)
    m = r.astype(ml_dtypes.bfloat16)
    l = (r - m.astype(np.float32)).astype(ml_dtypes.bfloat16)
    return h, m, l


def _query_feats(p):
    n2 = (p * p).sum(1, keepdims=True)
    one = np.ones((len(p), 1), np.float32)
    return np.concatenate([p, n2, one], 1).astype(np.float32)       # [n, 5]


def _ref_feats(p):
    n2 = (p * p).sum(1, keepdims=True)
    one = np.ones((len(p), 1), np.float32)
    return np.concatenate([-2.0 * p, one, n2], 1).astype(np.float32)  # [n, 5]


def _lift(fa, gb):
    """[n,5] fp32 pairs -> K=30 bf16 rows so that aT.T @ gT ~= fa @ gb.T."""
    ah, am, al = _split3(fa)
    bh, bm, bl = _split3(gb)
    aT = np.concatenate([ah, ah, ah, am, am, al], 1).T.copy()  # [30, n]
    gT = np.concatenate([bh, bm, bl, bh, bm, bh], 1).T.copy()  # [30, n]
    return aT, gT


def _band_starts():
    t = np.arange(NBLK)
    return np.clip(P * t + P // 2 - W // 2, 0, N - W)


def _exact_rows(q, r, rows):
    """Exact min squared distance (fp64) for query rows `rows` against all refs."""
    out = np.empty(len(rows))
    r64 = r.astype(np.float64)
    CH = 2048
    for s in range(0, len(rows), CH):
        qq = q[rows[s:s + CH]].astype(np.float64)
        d = ((qq[:, None, :] - r64[None, :, :]) ** 2).sum(-1)
        out[s:s + CH] = d.min(1)
    return out


def _finish_job(md_strip, qs, rs):
    """md_strip [P, NBLK] device band-mins for sorted queries; verify + repair.
    Returns md for sorted queries [N] (float64 internally, fp32-exactness ok)."""
    md = md_strip.T.reshape(N).astype(np.float64)        # sorted-query order
    C = _band_starts()
    Ci = np.repeat(C, P)                                  # per query
    qx = qs[:, 0].astype(np.float64)
    rx = rs[:, 0].astype(np.float64)

    hasL = Ci > 0
    xL = rx[np.clip(Ci - 1, 0, N - 1)]
    bad_L = hasL & ((qx < xL) | (md + E_DEV > (qx - xL) ** 2))
    hasR = (Ci + W) < N
    xR = rx[np.clip(Ci + W, 0, N - 1)]
    bad_R = hasR & ((qx > xR) | (md + E_DEV > (xR - qx) ** 2))
    bad = np.flatnonzero(bad_L | bad_R)
    _CACHE.setdefault("repairs", []).append(len(bad))
    if len(bad):
        md[bad] = _exact_rows(qs, rs, bad)
    return md


def kernel(xyz1: np.ndarray, xyz2: np.ndarray):
    xyz1 = np.asarray(xyz1, dtype=np.float32)
    xyz2 = np.asarray(xyz2, dtype=np.float32)
    B = xyz1.shape[0]
    assert xyz1.shape == (B, N, 3) and xyz2.shape == (B, N, 3)

    # 8 jobs: (batch, direction). direction 0: queries=xyz1 refs=xyz2 -> dist1
    jobs = []
    for b in range(B):
        jobs.append((xyz1[b], xyz2[b]))
        jobs.append((xyz2[b], xyz1[b]))

    in_maps = []
    sorted_pts = []
    for (q, r) in jobs:
        oq = np.argsort(q[:, 0], kind="stable")
        orr = np.argsort(r[:, 0], kind="stable")
        qs, rs = q[oq], r[orr]
        aT, gT = _lift(_query_feats(qs), _ref_feats(rs))
        in_maps.append({"aT": aT, "gT": gT})
        sorted_pts.append((qs, rs, oq))

    nc = _get_nc()
    _CACHE["last_in_maps"] = in_maps
    res = run_bass_kernel_spmd(nc, in_maps, core_ids=list(range(len(jobs))))
    _CACHE["last_results"] = res

    dist1 = np.empty((B, N), np.float32)
    dist2 = np.empty((B, N), np.float32)
    for j, (qs, rs, oq) in enumerate(sorted_pts):
        md_sorted = _finish_job(res.results[j]["md"], qs, rs)
        md = np.empty(N, np.float64)
        md[oq] = md_sorted
        if j % 2 == 0:
            dist1[j // 2] = md.astype(np.float32)
        else:
            dist2[j // 2] = md.astype(np.float32)
    return dist1, dist2

